# revision 17
# baseline (speedup 1.0000x reference)
"""AllGather MoE grouped-GEMM kernel for 8 TRN2 NeuronCores.

Strategy (tensor-parallel over the intermediate dim):
  - Each core holds a shard of tokens [M/8, K] (f32) and the full weight's
    N-slice for its rank: gate cols [128r,128r+128) and up cols
    [1024+128r, 1024+128r+128) of up_weight -> [E, K, 256] (bf16, host-cast).
  - Device: cast local tokens f32->bf16, AllGather -> H [M, K] bf16 in DRAM.
  - Host computes a routing plan: tokens sorted into 128-token tiles where all
    tokens of a tile share the same (unordered) expert pair {a, b}; diagonal
    (a==a) tokens fill tail slack of {a, x} tiles (their a-result is used for
    both topk slots). Residual slots padded with token 0 (output discarded).
  - Device per tile: indirect-DMA row gather of the tile's tokens, xbar DMA
    transpose to put K on partitions, then matmul with the token tile
    stationary and [W_a | W_b] streaming -> PSUM [128, 2, 2, 128]
    ([expert, gate/up, n]); fused SiLU(gate)*up epilogue -> out rows.
  - Host unpermutes rows and concatenates the 8 column slices.
"""

import os
import sys
import time
from dataclasses import dataclass

import numpy as np

for _p in ("/opt/trn_rl_repo", "/root/.axon_site/_ro/trn_rl_repo"):
    if os.path.isdir(_p) and _p not in sys.path:
        sys.path.insert(0, _p)

import ml_dtypes  # noqa: E402

P = 128  # partitions / tile token count


@dataclass(frozen=True)
class Cfg:
    M: int = 16384      # total tokens
    K: int = 1024       # hidden dim
    E: int = 8          # experts
    N: int = 2048       # fused gate+up intermediate (full)
    TOPK: int = 2
    R: int = 8          # cores
    T_BLK: int = 8      # token-tiles per pipeline block

    @property
    def ML(self):  # local tokens per core
        return self.M // self.R

    @property
    def KC(self):  # K chunks of 128
        return self.K // P

    @property
    def NPR(self):  # N columns per rank (gate half + up half)
        return self.N // self.R

    @property
    def NH(self):  # gate (or up) width per rank
        return self.NPR // 2


DEFAULT_CFG = Cfg()


# ---------------------------------------------------------------------------
# Host-side routing plan
# ---------------------------------------------------------------------------

def plan_routing(ids: np.ndarray, cfg: Cfg):
    """Sort tokens into 128-token tiles of homogeneous expert pairs.

    Returns dict with:
      slots      [n_tiles*P] int32: token id per slot (dummy slots hold 0)
      tile_pairs [n_tiles, 2] int: (a, b) expert pair per tile, a <= b
      pos        [M] int64: slot position (tile*P + lane) of each token
    """
    M, E = cfg.M, cfg.E
    a = np.minimum(ids[:, 0], ids[:, 1]).astype(np.int64)
    b = np.maximum(ids[:, 0], ids[:, 1]).astype(np.int64)

    diag = [list(np.nonzero((a == e) & (b == e))[0]) for e in range(E)]
    slots: list[int] = []
    used: list[bool] = []
    tile_pairs: list[tuple[int, int]] = []

    def emit_tile(toks, pa, pb):
        t = list(toks)
        assert len(t) <= P
        pad = P - len(t)
        slots.extend(t + [0] * pad)
        used.extend([True] * len(t) + [False] * pad)
        tile_pairs.append((pa, pb))

    for pa in range(E):
        for pb in range(pa + 1, E):
            toks = list(np.nonzero((a == pa) & (b == pb))[0])
            if not toks:
                continue
            # fill tail slack with diagonal tokens of expert pa or pb
            slack = (-len(toks)) % P
            take_a = min(slack, len(diag[pa]))
            toks += diag[pa][:take_a]
            diag[pa] = diag[pa][take_a:]
            slack -= take_a
            take_b = min(slack, len(diag[pb]))
            toks += diag[pb][:take_b]
            diag[pb] = diag[pb][take_b:]
            for i in range(0, len(toks), P):
                emit_tile(toks[i:i + P], pa, pb)
    for e in range(E):
        for i in range(0, len(diag[e]), P):
            emit_tile(diag[e][i:i + P], e, e)

    slots_arr = np.asarray(slots, dtype=np.int32)
    used_arr = np.asarray(used, dtype=bool)
    pos = np.empty(M, dtype=np.int64)
    pos[slots_arr[used_arr]] = np.nonzero(used_arr)[0]
    return {
        "slots": slots_arr,
        "tile_pairs": np.asarray(tile_pairs, dtype=np.int64),
        "pos": pos,
        "n_tiles": len(tile_pairs),
    }


# ---------------------------------------------------------------------------
# Device graph
# ---------------------------------------------------------------------------

def _enable_ldw_opt():
    """walrus is invoked with --enable-ldw-opt=false; flip it (validated by
    the rel-err check — fast-weight-load halves LDWEIGHTS time)."""
    from concourse import bass_utils
    if getattr(bass_utils.run_command, "_ldw_patched", False):
        return
    orig = bass_utils.run_command

    def patched(argv, **kw):
        argv = ["--enable-ldw-opt=true" if a == "--enable-ldw-opt=false" else a
                for a in argv]
        return orig(argv, **kw)

    patched._ldw_patched = True
    bass_utils.run_command = patched


def build_graph(cfg: Cfg, n_tiles: int, tile_pairs: np.ndarray, debug=False):
    from concourse import bacc, bass, mybir
    import concourse.tile as tile

    f32, bf16, i32 = mybir.dt.float32, mybir.dt.bfloat16, mybir.dt.int32
    ML, K, KC, NPR, NH, E = cfg.ML, cfg.K, cfg.KC, cfg.NPR, cfg.NH, cfg.E
    RG = [list(range(cfg.R))]

    nc = bacc.Bacc("TRN2", target_bir_lowering=False, debug=False,
                   num_devices=cfg.R)
    h_in = nc.dram_tensor("h", [ML, K], f32, kind="ExternalInput")
    w_in = nc.dram_tensor("w", [P, E, KC, NPR], bf16, kind="ExternalInput")
    idx_in = nc.dram_tensor("idx", [P, n_tiles], i32, kind="ExternalInput")
    out_ext = nc.dram_tensor("out", [P, n_tiles, NPR], bf16,
                             kind="ExternalOutput")
    if debug:
        dbg_h = nc.dram_tensor("dbg_h", [cfg.M, K], bf16,
                               kind="ExternalOutput")
        dbg_a = nc.dram_tensor("dbg_a", [P, cfg.T_BLK * K], bf16,
                               kind="ExternalOutput")
        dbg_at = nc.dram_tensor("dbg_at", [P, cfg.T_BLK * KC, P], bf16,
                                kind="ExternalOutput")

    with tile.TileContext(nc) as tc:
        with (
            tc.tile_pool(name="dram", bufs=1, space="DRAM") as dpool,
            tc.tile_pool(name="persist", bufs=1) as pers,
            tc.tile_pool(name="cast", bufs=3) as cp,
            tc.tile_pool(name="gat", bufs=2) as gp,
            tc.tile_pool(name="att", bufs=2) as tp,
            tc.tile_pool(name="psum", bufs=4, space="PSUM") as psp,
            tc.tile_pool(name="sil", bufs=4) as slp,
            tc.tile_pool(name="osb", bufs=2) as op_,
        ):
            ag_in = dpool.tile([ML, K], bf16, name="ag_in")
            ag_out = dpool.tile([cfg.M, K], bf16, addr_space="Shared",
                                name="ag_out")
            w_sb = pers.tile([P, E, KC, NPR], bf16, name="w_sb")
            idx_sb = pers.tile([P, n_tiles], i32, name="idx_sb")
            nc.scalar.dma_start(out=w_sb[:], in_=w_in[:, :, :, :])
            nc.scalar.dma_start(out=idx_sb[:], in_=idx_in[:, :])

            # PE warmup: dummy matmuls during the cast+AllGather head so the
            # PE clock has ramped to full pstate when real matmuls arrive.
            warm_a = pers.tile([P, P], bf16, name="warm_a")
            warm_b = pers.tile([P, 512], bf16, name="warm_b")
            nc.vector.memset(warm_a[:], 0)
            nc.vector.memset(warm_b[:], 0)
            for _wi in range(48):
                warm_ps = psp.tile([P, 2, 2, NH], f32, name="warm_ps",
                                   tag="ps2")
                nc.tensor.matmul(warm_ps[:], warm_a[:],
                                 warm_b[:, :4 * NH],
                                 start=True, stop=True)

            # cast local shard f32 -> bf16 into the all-gather input bounce
            for i in range(ML // P):
                t32 = cp.tile([P, K], f32, name="t32", tag="t32")
                nc.scalar.dma_start(out=t32[:], in_=h_in[i * P:(i + 1) * P, :])
                t16 = cp.tile([P, K], bf16, name="t16", tag="t16")
                nc.vector.tensor_copy(out=t16[:], in_=t32[:])
                nc.scalar.dma_start(out=ag_in[i * P:(i + 1) * P, :], in_=t16[:])

            nc.gpsimd.collective_compute(
                "AllGather", mybir.AluOpType.bypass, replica_groups=RG,
                ins=[ag_in.opt()], outs=[ag_out.opt()],
            )
            if debug:
                nc.sync.dma_start(out=dbg_h[:, :], in_=ag_out[:])

            TB = cfg.T_BLK
            n_blocks = (n_tiles + TB - 1) // TB
            for blk in range(n_blocks):
                g0 = blk * TB
                tb_sz = min(TB, n_tiles - g0)
                a_t = gp.tile([P, tb_sz * K], bf16, name="a_t", tag="a_t")
                for j in range(tb_sz):
                    # HW indirect DMA consumes ONE index per partition row
                    nc.gpsimd.indirect_dma_start(
                        out=a_t[:, j * K:(j + 1) * K], out_offset=None,
                        in_=ag_out[:, :],
                        in_offset=bass.IndirectOffsetOnAxis(
                            ap=idx_sb[:, g0 + j:g0 + j + 1], axis=0),
                    )
                at_t = tp.tile([P, tb_sz * KC, P], bf16, name="at_t",
                               tag="at_t")
                nc.sync.dma_start_transpose(out=at_t[:], in_=a_t[:])
                if debug and blk == 0:
                    nc.sync.dma_start(out=dbg_a[:, :], in_=a_t[:])
                    nc.sync.dma_start(out=dbg_at[:, :, :], in_=at_t[:])
                o_sb = op_.tile([P, tb_sz, NPR], bf16, name="o_sb", tag="o_sb")
                for j in range(tb_sz):
                    g = g0 + j
                    pa, pb = int(tile_pairs[g, 0]), int(tile_pairs[g, 1])
                    nh = 2 if pa != pb else 1
                    ps = psp.tile([P, nh, 2, NH], f32, name="ps",
                                  tag=f"ps{nh}")
                    for c in range(KC):
                        lhsT = at_t[:, j * KC + c, :]
                        if nh == 2:
                            rhs = w_sb[:, pa:pb + 1:(pb - pa), c, :]
                        else:
                            rhs = w_sb[:, pa, c, :]
                        nc.tensor.matmul(ps[:], lhsT, rhs,
                                         start=(c == 0), stop=(c == KC - 1))
                    sil = slp.tile([P, nh, NH], f32, name="sil",
                                   tag=f"sil{nh}")
                    nc.scalar.activation(
                        out=sil[:], in_=ps[:, :, 0, :],
                        func=mybir.ActivationFunctionType.Silu)
                    nc.vector.tensor_tensor(
                        out=o_sb[:, j, 0:nh * NH], in0=sil[:],
                        in1=ps[:, :, 1, :], op=mybir.AluOpType.mult)
                nc.scalar.dma_start(out=out_ext[:, g0:g0 + tb_sz, :],
                                    in_=o_sb[:])
    nc.compile()
    return nc


# ---------------------------------------------------------------------------
# Host-side input prep / output assembly
# ---------------------------------------------------------------------------

def make_in_maps(local_hidden_states, up_weight, plan, cfg: Cfg):
    h = np.ascontiguousarray(np.asarray(local_hidden_states, dtype=np.float32))
    w = np.asarray(up_weight, dtype=np.float32)
    n_tiles = plan["n_tiles"]
    # idx [P, n_tiles]: column g = tokens of tile g
    idx = np.ascontiguousarray(
        plan["slots"].reshape(n_tiles, P).T.astype(np.int32))
    Nhalf = cfg.N // 2
    in_maps = []
    for r in range(cfg.R):
        gate = w[:, :, cfg.NH * r:cfg.NH * (r + 1)]
        up = w[:, :, Nhalf + cfg.NH * r:Nhalf + cfg.NH * (r + 1)]
        wr = np.concatenate([gate, up], axis=2)  # [E, K, NPR]
        # -> [P(k%128), E, KC, NPR]
        wr = wr.reshape(cfg.E, cfg.KC, P, cfg.NPR).transpose(2, 0, 1, 3)
        wr = np.ascontiguousarray(wr.astype(ml_dtypes.bfloat16))
        in_maps.append({
            "h": h[cfg.ML * r:cfg.ML * (r + 1), :],
            "w": wr,
            "idx": idx,
        })
    return in_maps


def assemble_output(core_outs, ids, plan, cfg: Cfg):
    """core_outs: list of R arrays [P, n_tiles, NPR] f32 -> [M*TOPK, N//2]."""
    n_tiles = plan["n_tiles"]
    pos = plan["pos"]                       # [M] slot position per token
    pair_a = plan["tile_pairs"][:, 0]       # [n_tiles]
    tile_of = pos // P                      # [M]
    rowpos = (pos // P) * P + (pos % P)     # == pos; row in (tile, lane) order

    # half selector per assignment row p = t*TOPK + k
    ids64 = np.asarray(ids, dtype=np.int64)
    half = (ids64 != pair_a[tile_of][:, None]).astype(np.int64)  # [M, TOPK]
    rows = np.repeat(rowpos, cfg.TOPK)      # [M*TOPK]
    halves = half.reshape(-1)               # [M*TOPK]

    cols = []
    for r in range(cfg.R):
        o = np.asarray(core_outs[r], dtype=np.float32)  # [P, n_tiles, NPR]
        # -> rows in (tile, lane) order: [n_tiles*P, 2, NH]
        blk = o.transpose(1, 0, 2).reshape(n_tiles * P, 2, cfg.NH)
        cols.append(blk[rows, halves, :])   # [M*TOPK, NH]
    return np.concatenate(cols, axis=1)


# ---------------------------------------------------------------------------
# Runners
# ---------------------------------------------------------------------------

def run_on_hw(nc, in_maps, cfg: Cfg, trace=False):
    from concourse.bass_utils import run_bass_kernel_spmd
    res = run_bass_kernel_spmd(nc, in_maps, core_ids=list(range(cfg.R)),
                               trace=trace)
    return list(res.results), res


def run_on_sim(nc, in_maps, cfg: Cfg):
    from concourse import bass_interp
    sim = bass_interp.MultiCoreSim(nc, cfg.R)
    for r in range(cfg.R):
        for k, v in in_maps[r].items():
            sim.cores[r].tensor(k)[:] = v
    sim.simulate(check_with_hw=False)
    outs = [{"out": np.array(sim.cores[r].tensor("out"))} for r in range(cfg.R)]
    return outs, None


def moe_kernel(local_hidden_states, up_weight, full_topk_ids, cfg: Cfg,
               runner="hw", trace=False, verbose=False, debug=False):
    ids = np.asarray(full_topk_ids)
    t0 = time.time()
    plan = plan_routing(ids, cfg)
    in_maps = make_in_maps(local_hidden_states, up_weight, plan, cfg)
    t1 = time.time()
    nc = build_graph(cfg, plan["n_tiles"], plan["tile_pairs"], debug=debug)
    t2 = time.time()
    if verbose:
        print(f"[kernel] plan+prep {t1-t0:.1f}s  build+compile {t2-t1:.1f}s  "
              f"n_tiles={plan['n_tiles']}", flush=True)
    if runner == "hw":
        outs, res = run_on_hw(nc, in_maps, cfg, trace=trace)
    else:
        outs, res = run_on_sim(nc, in_maps, cfg)
    t3 = time.time()
    if verbose:
        print(f"[kernel] run {t3-t2:.1f}s", flush=True)
    moe_kernel.last_outs = outs
    moe_kernel.last_plan = plan
    moe_kernel.last_in_maps = in_maps
    out = assemble_output([o["out"] for o in outs], ids, plan, cfg)
    if verbose and res is not None:
        print(f"[kernel] exec_time_ns={res.exec_time_ns}", flush=True)
    moe_kernel.last_result = res
    return out.astype(np.float32)


def kernel(local_hidden_states, up_weight, full_topk_ids):
    return moe_kernel(local_hidden_states, up_weight, full_topk_ids,
                      DEFAULT_CFG, runner="hw")


# revision 74
# speedup vs baseline: 1.2348x; 1.2348x over previous
"""AllGather MoE grouped-GEMM kernel for 8 TRN2 NeuronCores.

Strategy (tensor-parallel over the intermediate dim):
  - Each core holds a shard of tokens [M/8, K] (f32) and the full weight's
    N-slice for its rank: gate cols [128r,128r+128) and up cols
    [1024+128r, 1024+128r+128) of up_weight -> [E, K, 256] (bf16, host-cast).
  - Device: cast local tokens f32->bf16, AllGather -> H [M, K] bf16 in DRAM.
  - Host computes a routing plan: tokens sorted into 128-token tiles where all
    tokens of a tile share the same (unordered) expert pair {a, b}; diagonal
    (a==a) tokens fill tail slack of {a, x} tiles (their a-result is used for
    both topk slots). Residual slots padded with token 0 (output discarded).
  - Device per tile: indirect-DMA row gather of the tile's tokens, xbar DMA
    transpose to put K on partitions, then matmul with the token tile
    stationary and [W_a | W_b] streaming -> PSUM [128, 2, 2, 128]
    ([expert, gate/up, n]); fused SiLU(gate)*up epilogue -> out rows.
  - Host unpermutes rows and concatenates the 8 column slices.
"""

import os
import sys
import time
from dataclasses import dataclass

import numpy as np

for _p in ("/opt/trn_rl_repo", "/root/.axon_site/_ro/trn_rl_repo"):
    if os.path.isdir(_p) and _p not in sys.path:
        sys.path.insert(0, _p)

import ml_dtypes  # noqa: E402

P = 128  # partitions / tile token count
XBAR_TILES = 4     # tiles per whole-tile xbar transpose op
XB_OPS = 1         # xbar transpose ops per block; rest of tiles via PE


@dataclass(frozen=True)
class Cfg:
    M: int = 16384      # total tokens
    K: int = 1024       # hidden dim
    E: int = 8          # experts
    N: int = 2048       # fused gate+up intermediate (full)
    TOPK: int = 2
    R: int = 8          # cores
    T_BLK: int = 8      # token-tiles per pipeline block
    Q: int = 1          # all-gather pieces (2+ collectives hit a ~1.8ms
                        # ncfw stall before the mesh algo starts; keep 1)

    @property
    def ML(self):  # local tokens per core
        return self.M // self.R

    @property
    def KC(self):  # K chunks of 128
        return self.K // P

    @property
    def NPR(self):  # N columns per rank (gate half + up half)
        return self.N // self.R

    @property
    def NH(self):  # gate (or up) width per rank
        return self.NPR // 2


DEFAULT_CFG = Cfg()


# ---------------------------------------------------------------------------
# Host-side routing plan
# ---------------------------------------------------------------------------

def plan_routing(ids: np.ndarray, cfg: Cfg):
    """Sort tokens into 128-token tiles of homogeneous expert pairs.

    Returns dict with:
      slots      [n_tiles*P] int32: token id per slot (dummy slots hold 0)
      tile_pairs [n_tiles, 2] int: (a, b) expert pair per tile, a <= b
      pos        [M] int64: slot position (tile*P + lane) of each token
    """
    M, E, Q = cfg.M, cfg.E, cfg.Q
    ML, PR = cfg.ML, cfg.ML // cfg.Q
    a = np.minimum(ids[:, 0], ids[:, 1]).astype(np.int64)
    b = np.maximum(ids[:, 0], ids[:, 1]).astype(np.int64)
    piece = np.arange(M) % ML // PR  # AG piece of each token

    # per (expert-pair, piece) token queues; tiles are piece-pure so each
    # tile gathers from exactly one all-gather output tensor
    diag = [[list(np.nonzero((a == e) & (b == e) & (piece == q))[0])
             for q in range(Q)] for e in range(E)]
    per_class: list[list] = [[] for _ in range(Q)]  # (tokens, pa, pb)

    def emit_tiles(toks, pa, pb, q):
        for i in range(0, len(toks), P):
            per_class[q].append((toks[i:i + P], pa, pb))

    for pa in range(E):
        for pb in range(pa + 1, E):
            for q in range(Q):
                toks = list(np.nonzero((a == pa) & (b == pb)
                                       & (piece == q))[0])
                if not toks:
                    continue
                # fill tail slack with same-piece diagonal tokens
                slack = (-len(toks)) % P
                take_a = min(slack, len(diag[pa][q]))
                toks += diag[pa][q][:take_a]
                diag[pa][q] = diag[pa][q][take_a:]
                slack -= take_a
                take_b = min(slack, len(diag[pb][q]))
                toks += diag[pb][q][:take_b]
                diag[pb][q] = diag[pb][q][take_b:]
                emit_tiles(toks, pa, pb, q)
    for e in range(E):
        for q in range(Q):
            if diag[e][q]:
                emit_tiles(diag[e][q], e, e, q)

    slots: list[int] = []
    used: list[bool] = []
    tile_pairs: list[tuple[int, int]] = []
    tile_class: list[int] = []
    for q in range(Q):  # class-0 tiles first: they only need AG piece 0
        for toks, pa, pb in per_class[q]:
            t = list(toks)
            pad = P - len(t)
            slots.extend(t + [0] * pad)
            used.extend([True] * len(t) + [False] * pad)
            tile_pairs.append((pa, pb))
            tile_class.append(q)

    flat_slots = np.asarray(slots, dtype=np.int64)
    flat_used = np.asarray(used, dtype=bool)
    pairs_arr = np.asarray(tile_pairs, dtype=np.int64)
    class_arr = np.asarray(tile_class, dtype=np.int64)
    pos = np.empty(M, dtype=np.int64)
    pos[flat_slots[flat_used]] = np.nonzero(flat_used)[0]

    # gather row within the piece's ag_out tensor [R*PR, K]
    rank = flat_slots // ML
    off = flat_slots % ML
    dev_rows = (rank * PR + (off % PR)).astype(np.int32)
    return {
        "slots": flat_slots,
        "dev_rows": dev_rows,
        "tile_pairs": pairs_arr,
        "tile_class": class_arr,
        "pos": pos,
        "n_tiles": len(pairs_arr),
    }


# ---------------------------------------------------------------------------
# Device graph
# ---------------------------------------------------------------------------

def _enable_ldw_opt():
    """walrus is invoked with --enable-ldw-opt=false; flip it (validated by
    the rel-err check — fast-weight-load halves LDWEIGHTS time)."""
    from concourse import bass_utils
    if getattr(bass_utils.run_command, "_ldw_patched", False):
        return
    orig = bass_utils.run_command

    def patched(argv, **kw):
        argv = ["--enable-ldw-opt=true" if a == "--enable-ldw-opt=false" else a
                for a in argv]
        return orig(argv, **kw)

    patched._ldw_patched = True
    bass_utils.run_command = patched


def build_graph(cfg: Cfg, n_tiles: int, tile_pairs: np.ndarray,
                tile_class=None, debug=False):
    from concourse import bacc, bass, mybir
    import concourse.tile as tile

    f32, bf16, i32 = mybir.dt.float32, mybir.dt.bfloat16, mybir.dt.int32
    ML, K, KC, NPR, NH, E = cfg.ML, cfg.K, cfg.KC, cfg.NPR, cfg.NH, cfg.E
    RG = [list(range(cfg.R))]
    Q, MQ, PRL = cfg.Q, cfg.M // cfg.Q, cfg.ML // cfg.Q
    if tile_class is None:
        tile_class = np.full(n_tiles, Q - 1, dtype=np.int64)

    nc = bacc.Bacc("TRN2", target_bir_lowering=False, debug=False,
                   num_devices=cfg.R)
    h_in = nc.dram_tensor("h", [ML, K], f32, kind="ExternalInput")
    w_in = nc.dram_tensor("w", [P, E, KC, NPR], bf16, kind="ExternalInput")
    idx_in = nc.dram_tensor("idx", [P, n_tiles], i32, kind="ExternalInput")
    out_ext = nc.dram_tensor("out", [P, n_tiles, NPR], bf16,
                             kind="ExternalOutput")
    if debug:
        dbg_h = nc.dram_tensor("dbg_h", [cfg.M, K], bf16,
                               kind="ExternalOutput")
        dbg_a = nc.dram_tensor("dbg_a", [P, cfg.T_BLK * K], bf16,
                               kind="ExternalOutput")
        dbg_at = nc.dram_tensor("dbg_at", [P, cfg.T_BLK * KC, P], bf16,
                                kind="ExternalOutput")

    with tile.TileContext(nc) as tc:
        with (
            tc.tile_pool(name="dram", bufs=1, space="DRAM") as dpool,
            tc.tile_pool(name="persist", bufs=1) as pers,
            tc.tile_pool(name="cast", bufs=3) as cp,
            tc.tile_pool(name="gat", bufs=2) as gp,
            tc.tile_pool(name="att", bufs=2) as tp,
            tc.tile_pool(name="psum", bufs=4, space="PSUM") as psp,
            tc.tile_pool(name="sil", bufs=4) as slp,
            tc.tile_pool(name="osb", bufs=2) as op_,
        ):
            ag_in = dpool.tile([ML, K], bf16, name="ag_in")
            ag_outs = [
                dpool.tile([cfg.M // Q, K], bf16, addr_space="Shared",
                           name=f"ag_out{qi}", uniquify=True)
                for qi in range(Q)
            ]
            w_sb = pers.tile([P, E, KC, NPR], bf16, name="w_sb")
            idx_sb = pers.tile([P, n_tiles], i32, name="idx_sb")
            # cast local shard f32 -> bf16 (split across both HWDGE rings
            # and both DVE/ACT so the all-gather can start early)
            ci = 0
            for qi in range(Q):
                for r0 in range(qi * PRL, (qi + 1) * PRL, P):
                    rows = min(P, (qi + 1) * PRL - r0)
                    eng = nc.sync if ci % 2 == 0 else nc.scalar
                    t32 = cp.tile([P, K], f32, name="t32", tag="t32")
                    eng.dma_start(out=t32[:rows], in_=h_in[r0:r0 + rows, :])
                    t16 = cp.tile([P, K], bf16, name="t16", tag="t16")
                    if ci % 2 == 0:
                        nc.vector.tensor_copy(out=t16[:rows], in_=t32[:rows])
                    else:
                        nc.scalar.activation(
                            out=t16[:rows], in_=t32[:rows],
                            func=mybir.ActivationFunctionType.Copy)
                    eng.dma_start(out=ag_in[r0:r0 + rows, :], in_=t16[:rows])
                    ci += 1
                nc.gpsimd.collective_compute(
                    "AllGather", mybir.AluOpType.bypass, replica_groups=RG,
                    ins=[ag_in[qi * PRL:(qi + 1) * PRL, :].opt()],
                    outs=[ag_outs[qi].opt()],
                )
            nc.scalar.dma_start(out=w_sb[:], in_=w_in[:, :, :, :])
            nc.scalar.dma_start(out=idx_sb[:], in_=idx_in[:, :])
            ident = pers.tile([P, P], bf16, name="ident")
            from concourse.masks import make_identity
            make_identity(nc, ident[:])

            if debug:
                nc.sync.dma_start(out=dbg_h[:, :], in_=ag_out[:])

            TB = cfg.T_BLK
            n_blocks = (n_tiles + TB - 1) // TB
            for blk in range(n_blocks):
                g0 = blk * TB
                tb_sz = min(TB, n_tiles - g0)
                a_t = gp.tile([P, tb_sz * K], bf16, name="a_t", tag="a_t")
                for j in range(tb_sz):
                    # HW indirect DMA consumes ONE index per partition
                    # row; each tile is piece-pure, so it gathers from
                    # the ag_out tensor of its class
                    cls = int(tile_class[g0 + j])
                    nc.gpsimd.indirect_dma_start(
                        out=a_t[:, j * K:(j + 1) * K], out_offset=None,
                        in_=ag_outs[cls][:, :],
                        in_offset=bass.IndirectOffsetOnAxis(
                            ap=idx_sb[:, g0 + j:g0 + j + 1], axis=0),
                    )
                # Hybrid transpose: the first XBAR_TILES tiles go through the
                # xbar (one DMA op, prefix destination slice - offset xbar
                # destinations corrupt data); the rest go through the PE with
                # an ACT/DVE copyback. Keeps the per-block DMA chain under
                # the matmul time.
                at_t = tp.tile([P, tb_sz * KC, P], bf16, name="at_t",
                               tag="at_t")
                xb = min(XBAR_TILES, tb_sz)
                nc.sync.dma_start_transpose(
                    out=at_t[:, :xb * KC, :], in_=a_t[:, :xb * K])
                for j in range(xb, tb_sz):
                    for c in range(KC):
                        ps_t = psp.tile([P, P], bf16, name="ps_t",
                                        tag="tp", bufs=4)
                        nc.tensor.transpose(
                            out=ps_t[:],
                            in_=a_t[:, j * K + c * P:j * K + (c + 1) * P],
                            identity=ident[:])
                        if c % 2 == 0:
                            nc.scalar.activation(
                                out=at_t[:, j * KC + c, :], in_=ps_t[:],
                                func=mybir.ActivationFunctionType.Copy)
                        else:
                            nc.vector.tensor_copy(
                                out=at_t[:, j * KC + c, :], in_=ps_t[:])

                def lhsT_of(j, c):
                    return at_t[:, j * KC + c, :]
                o_sb = op_.tile([P, tb_sz, NPR], bf16, name="o_sb", tag="o_sb")
                for j in range(tb_sz):
                    g = g0 + j
                    pa, pb = int(tile_pairs[g, 0]), int(tile_pairs[g, 1])
                    nh = 2 if pa != pb else 1
                    ps_full = psp.tile([P, 2, 2, NH], f32, name="ps",
                                       tag="ps2")
                    ps = ps_full[:, :nh, :, :] if nh == 1 else ps_full
                    for c in range(KC):
                        lhsT = lhsT_of(j, c)
                        if nh == 2:
                            rhs = w_sb[:, pa:pb + 1:(pb - pa), c, :]
                        else:
                            rhs = w_sb[:, pa, c, :]
                        nc.tensor.matmul(ps[:], lhsT, rhs,
                                         start=(c == 0), stop=(c == KC - 1))
                    sil = slp.tile([P, nh, NH], f32, name="sil",
                                   tag=f"sil{nh}")
                    nc.scalar.activation(
                        out=sil[:], in_=ps[:, :, 0, :],
                        func=mybir.ActivationFunctionType.Silu)
                    nc.vector.tensor_tensor(
                        out=o_sb[:, j, 0:nh * NH], in0=sil[:],
                        in1=ps[:, :, 1, :], op=mybir.AluOpType.mult)
                nc.sync.dma_start(out=out_ext[:, g0:g0 + tb_sz, :],
                                  in_=o_sb[:])
    nc.compile()
    return nc


# ---------------------------------------------------------------------------
# Host-side input prep / output assembly
# ---------------------------------------------------------------------------

def make_in_maps(local_hidden_states, up_weight, plan, cfg: Cfg):
    h = np.ascontiguousarray(np.asarray(local_hidden_states, dtype=np.float32))
    w = np.asarray(up_weight, dtype=np.float32)
    n_tiles = plan["n_tiles"]
    # idx [P, n_tiles]: column g = device gather rows of tile g
    idx = np.ascontiguousarray(
        plan["dev_rows"].reshape(n_tiles, P).T.astype(np.int32))
    Nhalf = cfg.N // 2
    in_maps = []
    for r in range(cfg.R):
        gate = w[:, :, cfg.NH * r:cfg.NH * (r + 1)]
        up = w[:, :, Nhalf + cfg.NH * r:Nhalf + cfg.NH * (r + 1)]
        wr = np.concatenate([gate, up], axis=2)  # [E, K, NPR]
        # -> [P(k%128), E, KC, NPR]
        wr = wr.reshape(cfg.E, cfg.KC, P, cfg.NPR).transpose(2, 0, 1, 3)
        wr = np.ascontiguousarray(wr.astype(ml_dtypes.bfloat16))
        in_maps.append({
            "h": h[cfg.ML * r:cfg.ML * (r + 1), :],
            "w": wr,
            "idx": idx,
        })
    return in_maps


def assemble_output(core_outs, ids, plan, cfg: Cfg):
    """core_outs: list of R arrays [P, n_tiles, NPR] f32 -> [M*TOPK, N//2]."""
    n_tiles = plan["n_tiles"]
    pos = plan["pos"]                       # [M] slot position per token
    pair_a = plan["tile_pairs"][:, 0]       # [n_tiles]
    tile_of = pos // P                      # [M]
    rowpos = (pos // P) * P + (pos % P)     # == pos; row in (tile, lane) order

    # half selector per assignment row p = t*TOPK + k
    ids64 = np.asarray(ids, dtype=np.int64)
    half = (ids64 != pair_a[tile_of][:, None]).astype(np.int64)  # [M, TOPK]
    rows = np.repeat(rowpos, cfg.TOPK)      # [M*TOPK]
    halves = half.reshape(-1)               # [M*TOPK]

    cols = []
    for r in range(cfg.R):
        o = np.asarray(core_outs[r], dtype=np.float32)  # [P, n_tiles, NPR]
        # -> rows in (tile, lane) order: [n_tiles*P, 2, NH]
        blk = o.transpose(1, 0, 2).reshape(n_tiles * P, 2, cfg.NH)
        cols.append(blk[rows, halves, :])   # [M*TOPK, NH]
    return np.concatenate(cols, axis=1)


# ---------------------------------------------------------------------------
# Runners
# ---------------------------------------------------------------------------

def run_on_hw(nc, in_maps, cfg: Cfg, trace=False):
    from concourse.bass_utils import run_bass_kernel_spmd
    res = run_bass_kernel_spmd(nc, in_maps, core_ids=list(range(cfg.R)),
                               trace=trace)
    return list(res.results), res


def run_on_sim(nc, in_maps, cfg: Cfg):
    from concourse import bass_interp
    sim = bass_interp.MultiCoreSim(nc, cfg.R)
    for r in range(cfg.R):
        for k, v in in_maps[r].items():
            sim.cores[r].tensor(k)[:] = v
    sim.simulate(check_with_hw=False)
    outs = [{"out": np.array(sim.cores[r].tensor("out"))} for r in range(cfg.R)]
    return outs, None


def moe_kernel(local_hidden_states, up_weight, full_topk_ids, cfg: Cfg,
               runner="hw", trace=False, verbose=False, debug=False):
    ids = np.asarray(full_topk_ids)
    t0 = time.time()
    plan = plan_routing(ids, cfg)
    in_maps = make_in_maps(local_hidden_states, up_weight, plan, cfg)
    t1 = time.time()
    nc = build_graph(cfg, plan["n_tiles"], plan["tile_pairs"],
                     tile_class=plan["tile_class"], debug=debug)
    t2 = time.time()
    if verbose:
        print(f"[kernel] plan+prep {t1-t0:.1f}s  build+compile {t2-t1:.1f}s  "
              f"n_tiles={plan['n_tiles']}", flush=True)
    if runner == "hw":
        outs, res = run_on_hw(nc, in_maps, cfg, trace=trace)
    else:
        outs, res = run_on_sim(nc, in_maps, cfg)
    t3 = time.time()
    if verbose:
        print(f"[kernel] run {t3-t2:.1f}s", flush=True)
    moe_kernel.last_outs = outs
    moe_kernel.last_plan = plan
    moe_kernel.last_in_maps = in_maps
    out = assemble_output([o["out"] for o in outs], ids, plan, cfg)
    if verbose and res is not None:
        print(f"[kernel] exec_time_ns={res.exec_time_ns}", flush=True)
    moe_kernel.last_result = res
    return out.astype(np.float32)


def kernel(local_hidden_states, up_weight, full_topk_ids):
    return moe_kernel(local_hidden_states, up_weight, full_topk_ids,
                      DEFAULT_CFG, runner="hw")
